# revision 1
# baseline (speedup 1.0000x reference)
"""BiLSTM-CRF Trainium2 kernel.

Strategy (2 of 8 cores do the heavy sequential work; the recurrence is
inherently serial per direction, so one core per direction):

  K1 (SPMD, cores 0..1; core0=forward, core1=backward via reversed inputs):
    - embedding gather (indirect DMA), PE transposes -> x^T
    - input projection xp^T = W_ihT^T... (fp32 matmuls), +bias, -> DRAM
    - 16384-step LSTM scan: W_hh in bf16 hi+lo split (bf16x2), h split
      hi+lo each step, fp32 PSUM accumulate -> near-fp32 accuracy at
      ~7.5us/step; output projection (w_out slice, bf16x2) fused into the
      scan -> partial feats pf (5, T)
  host: feats = pf_f + reverse(pf_b) + b_out
  Viterbi (host, fp32, same associativity as reference) + backtrace.

Self-contained: hardcodes all shapes from the problem spec.
"""
import os
import numpy as np
import ml_dtypes

V, E, H = 50000, 512, 1024
HH = H // 2           # 512 per direction
G = 4 * HH            # 2048 gates
KC = HH // 128        # 4 h-chunks
MC = G // 128         # 16 gate m-tiles
NT = 5
START, STOP = 3, 4
T = 16384
NEG = -10000.0
U = 16                # scan steps per For_i body
TB = 512              # phase-A T-block size

_cache = {}

# gate permutation: reference gate order is [i, f, g, o] in rows of w_ih/w_hh.
# device layout wants m-tiles ordered [i(0:4), f(4:8), o(8:12), g(12:16)].
def _perm():
    p = np.arange(G).reshape(4, HH)   # rows: i, f, g, o
    return np.concatenate([p[0], p[1], p[3], p[2]])  # i, f, o, g


def _build_k1():
    import concourse.bass as bass
    import concourse.mybir as mybir
    from concourse import bacc
    from concourse.bass import ds, ts
    from concourse.tile import TileContext
    from concourse.masks import make_identity

    f32 = mybir.dt.float32
    bf16 = mybir.dt.bfloat16
    u32 = mybir.dt.uint32

    nc = bacc.Bacc("TRN2", target_bir_lowering=False, debug=False, num_devices=2)

    emb_d = nc.dram_tensor("emb", [V, E], f32, kind="ExternalInput").ap()
    idx_d = nc.dram_tensor("idx", [128, T // 128], u32, kind="ExternalInput").ap()
    wihT_d = nc.dram_tensor("wihT", [KC, 128, G], f32, kind="ExternalInput").ap()
    bias_d = nc.dram_tensor("bias", [128, MC], f32, kind="ExternalInput").ap()
    whh_d = nc.dram_tensor("whh", [2 * KC, 128, G], bf16, kind="ExternalInput").ap()
    woT_d = nc.dram_tensor("woT", [2 * KC, 128, NT], bf16, kind="ExternalInput").ap()
    h0_d = nc.dram_tensor("h0v", [128, KC], f32, kind="ExternalInput").ap()
    c0_d = nc.dram_tensor("c0v", [128, KC], f32, kind="ExternalInput").ap()

    xpT_d = nc.dram_tensor("xpT", [MC, 128, T], f32, kind="Internal").ap()
    pf_d = nc.dram_tensor("pf", [NT, T], f32, kind="ExternalOutput").ap()

    with TileContext(nc) as tc:
        with (
            tc.tile_pool(name="const", bufs=1) as cpool,
            tc.tile_pool(name="wih", bufs=1) as wihpool,
            tc.tile_pool(name="whh", bufs=1) as whhpool,
            tc.tile_pool(name="state", bufs=1) as spool,
            tc.tile_pool(name="x", bufs=2) as xpool,
            tc.tile_pool(name="xt", bufs=2) as xtpool,
            tc.tile_pool(name="xpo", bufs=3) as xpopool,
            tc.tile_pool(name="xps", bufs=3) as xpspool,
            tc.tile_pool(name="pfs", bufs=3) as pfspool,
            tc.tile_pool(name="tmp", bufs=2) as tpool,
            tc.tile_pool(name="pst", bufs=2, space="PSUM") as pst,   # transposes
            tc.tile_pool(name="psx", bufs=2, space="PSUM") as psx,   # xp matmuls
            tc.tile_pool(name="psg", bufs=2, space="PSUM") as psg,   # scan gates
            tc.tile_pool(name="psf", bufs=2, space="PSUM") as psf,   # pf matmuls
        ):
            ident = cpool.tile([128, 128], f32, tag="ident")
            make_identity(nc, ident[:, :])
            idxt = cpool.tile([128, T // 128], u32, tag="idx")
            nc.sync.dma_start(out=idxt[:, :], in_=idx_d[:, :])
            biast = cpool.tile([128, MC], f32, tag="bias")
            nc.sync.dma_start(out=biast[:, :], in_=bias_d[:, :])

            wih = []
            for k in range(KC):
                w = wihpool.tile([128, G], f32, tag=f"wih{k}")
                nc.sync.dma_start(out=w[:, :], in_=wihT_d[k, :, :])
                wih.append(w)
            whh = []
            for k in range(2 * KC):
                w = whhpool.tile([128, G], bf16, tag=f"whh{k}")
                nc.sync.dma_start(out=w[:, :], in_=whh_d[k, :, :])
                whh.append(w)
            woT = cpool.tile([128, 2 * KC * NT], bf16, tag="woT")
            nc.sync.dma_start(
                out=woT[:, :].rearrange("p (w n) -> p w n", w=2 * KC),
                in_=woT_d[:, :, :].rearrange("w p n -> p w n"),
            )

            # ---------------- phase A: gather + transpose + xp ----------------
            for tb in range(T // TB):
                xts = []
                for e in range(KC):
                    xt = xtpool.tile([128, TB], f32, tag=f"xt{e}")
                    xts.append(xt)
                for sub in range(TB // 128):
                    b = tb * (TB // 128) + sub
                    xg = xpool.tile([128, E], f32, tag="xg")
                    nc.gpsimd.indirect_dma_start(
                        out=xg[:, :],
                        out_offset=None,
                        in_=emb_d[:, :],
                        in_offset=bass.IndirectOffsetOnAxis(ap=idxt[:, b:b + 1], axis=0),
                    )
                    for e in range(KC):
                        pt = pst.tile([128, 128], f32, tag="pt")
                        nc.tensor.transpose(pt[:, :], xg[:, ts(e, 128)], ident[:, :])
                        nc.vector.tensor_copy(xts[e][:, ts(sub, 128)], pt[:, :])
                for m in range(MC):
                    px = psx.tile([128, TB], f32, tag="px")
                    for e in range(KC):
                        nc.tensor.matmul(
                            px[:, :], wih[e][:, ts(m, 128)], xts[e][:, :],
                            start=(e == 0), stop=(e == KC - 1),
                        )
                    xo = xpopool.tile([128, TB], f32, tag="xo")
                    nc.vector.tensor_scalar(
                        out=xo[:, :], in0=px[:, :], scalar1=biast[:, m:m + 1],
                        scalar2=None, op0=mybir.AluOpType.add,
                    )
                    nc.sync.dma_start(out=xpT_d[m, :, ts(tb, TB)], in_=xo[:, :])

            # ---------------- scan ----------------
            c_t = spool.tile([128, KC], f32, tag="c")
            nc.sync.dma_start(out=c_t[:, :], in_=c0_d[:, :])
            hbuf = spool.tile([128, KC * (U + 1)], f32, tag="hbuf")
            nc.sync.dma_start(out=hbuf[:, 0:KC], in_=h0_d[:, :])
            hbf = spool.tile([128, 2 * KC * (U + 1)], bf16, tag="hbf")
            s0 = hbf[:, 0:2 * KC].rearrange("p (k two) -> p k two", two=2)
            h0r = hbuf[:, 0:KC].rearrange("p (k one) -> p k one", one=1)
            nc.vector.tensor_copy(s0[:, :, 0:1], h0r)
            tmp0 = tpool.tile([128, KC], f32, tag="hlo")
            tmp0r = tmp0[:, :].rearrange("p (k one) -> p k one", one=1)
            nc.vector.tensor_tensor(tmp0r, h0r, s0[:, :, 0:1], mybir.AluOpType.subtract)
            nc.vector.tensor_copy(s0[:, :, 1:2], tmp0r)

            with tc.For_i(0, T, U) as it:
                xpt = xpspool.tile([128, MC * U], f32, tag="xpt")
                nc.sync.dma_start(
                    out=xpt[:, :].rearrange("q (j t) -> q j t", j=MC),
                    in_=xpT_d[:, :, ds(it, U)].rearrange("j q t -> q j t"),
                )
                for s in range(U):
                    hin = hbf[:, 2 * KC * s:2 * KC * (s + 1)]
                    g = psg.tile([128, 2 * MC], f32, tag="g")
                    for m in range(MC):
                        for wv in range(2):
                            for k in range(KC):
                                nc.tensor.matmul(
                                    g[:, 2 * m:2 * m + 2],
                                    whh[wv * KC + k][:, ts(m, 128)],
                                    hin[:, 2 * k:2 * k + 2],
                                    start=(wv == 0 and k == 0),
                                    stop=(wv == 1 and k == KC - 1),
                                )
                    a = tpool.tile([128, MC], f32, tag="a")
                    gr = g[:, :].rearrange("p (m two) -> p m two", two=2)
                    ar = a[:, :].rearrange("p (m one) -> p m one", one=1)
                    xr = xpt[:, :].rearrange("q (j t) -> q j t", j=MC)[:, :, s:s + 1]
                    nc.vector.tensor_tensor(ar, gr[:, :, 0:1], xr, mybir.AluOpType.add)
                    nc.vector.tensor_tensor(ar, ar, gr[:, :, 1:2], mybir.AluOpType.add)
                    act = tpool.tile([128, MC], f32, tag="act")
                    nc.scalar.activation(act[:, 0:12], a[:, 0:12],
                                         mybir.ActivationFunctionType.Sigmoid)
                    nc.scalar.activation(act[:, 12:16], a[:, 12:16],
                                         mybir.ActivationFunctionType.Tanh)
                    ig = tpool.tile([128, KC], f32, tag="ig")
                    nc.vector.tensor_tensor(ig[:, :], act[:, 0:4], act[:, 12:16],
                                            mybir.AluOpType.mult)
                    nc.vector.tensor_tensor(c_t[:, :], act[:, 4:8], c_t[:, :],
                                            mybir.AluOpType.mult)
                    nc.vector.tensor_tensor(c_t[:, :], c_t[:, :], ig[:, :],
                                            mybir.AluOpType.add)
                    tct = tpool.tile([128, KC], f32, tag="tc")
                    nc.scalar.activation(tct[:, :], c_t[:, :],
                                         mybir.ActivationFunctionType.Tanh)
                    hout = hbuf[:, KC * (s + 1):KC * (s + 2)]
                    nc.vector.tensor_tensor(hout, act[:, 8:12], tct[:, :],
                                            mybir.AluOpType.mult)
                    sl = hbf[:, 2 * KC * (s + 1):2 * KC * (s + 2)].rearrange(
                        "p (k two) -> p k two", two=2)
                    hor = hout.rearrange("p (k one) -> p k one", one=1)
                    nc.vector.tensor_copy(sl[:, :, 0:1], hor)
                    hlo = tpool.tile([128, KC], f32, tag="hlo")
                    hlor = hlo[:, :].rearrange("p (k one) -> p k one", one=1)
                    nc.vector.tensor_tensor(hlor, hor, sl[:, :, 0:1],
                                            mybir.AluOpType.subtract)
                    nc.vector.tensor_copy(sl[:, :, 1:2], hlor)
                # fused output projection for these U steps
                pp = psf.tile([NT, 2 * U], f32, tag="pp")
                rhs = hbf[:, 2 * KC:].rearrange("p (s kk) -> p s kk", kk=2 * KC)
                for wv in range(2):
                    for k in range(KC):
                        nc.tensor.matmul(
                            pp[:, :],
                            woT[:, :].rearrange("p (w n) -> p w n", w=2 * KC)[:, wv * KC + k, :],
                            rhs[:, 0:U, 2 * k:2 * k + 2],
                            start=(wv == 0 and k == 0),
                            stop=(wv == 1 and k == KC - 1),
                        )
                pft = pfspool.tile([NT, 2 * U], f32, tag="pft")
                nc.vector.tensor_copy(pft[:, :], pp[:, :])
                pfo = pfspool.tile([NT, U], f32, tag="pfo")
                pfr = pft[:, :].rearrange("p (s two) -> p s two", two=2)
                nc.vector.tensor_tensor(
                    pfo[:, :].rearrange("p (s one) -> p s one", one=1),
                    pfr[:, :, 0:1], pfr[:, :, 1:2], mybir.AluOpType.add)
                nc.sync.dma_start(out=pf_d[:, ds(it, U)], in_=pfo[:, :])
                # carry h to slot 0
                nc.vector.tensor_copy(hbuf[:, 0:KC], hbuf[:, KC * U:KC * (U + 1)])
                nc.vector.tensor_copy(hbf[:, 0:2 * KC], hbf[:, 2 * KC * U:2 * KC * (U + 1)])
    nc.compile()
    return nc


def _prep_core(inputs, d):
    """Per-core (direction d: 0=fwd, 1=bwd) input dict for K1."""
    perm = _perm()
    sfx = "f" if d == 0 else "b"
    sent = np.asarray(inputs["sentence"]).astype(np.uint32)
    if d == 1:
        sent = sent[::-1].copy()
    idx = sent.reshape(T // 128, 128).T.copy()  # [p, b] = sent[b*128+p]

    w_ih = np.asarray(inputs[f"w_ih_{sfx}"], np.float32)[perm]     # (G, E)
    wihT = w_ih.T.reshape(KC, 128, G).copy()                        # [e,128,G]
    bias = (np.asarray(inputs[f"b_ih_{sfx}"], np.float32) +
            np.asarray(inputs[f"b_hh_{sfx}"], np.float32))[perm]
    bias = bias.reshape(MC, 128).T.copy()                           # (128, MC)

    w_hh = np.asarray(inputs[f"w_hh_{sfx}"], np.float32)[perm]      # (G, HH)
    whhT = w_hh.T.reshape(KC, 128, G)                               # [k,128,G]
    hi = whhT.astype(ml_dtypes.bfloat16)
    lo = (whhT - hi.astype(np.float32)).astype(ml_dtypes.bfloat16)
    whh = np.concatenate([hi, lo], axis=0).copy()                   # (8,128,G)

    wo = np.asarray(inputs["w_out"], np.float32)[:, d * HH:(d + 1) * HH]  # (5, HH)
    woT = wo.T.reshape(KC, 128, NT)                                  # [k,128,5]
    wohi = woT.astype(ml_dtypes.bfloat16)
    wolo = (woT - wohi.astype(np.float32)).astype(ml_dtypes.bfloat16)
    woTb = np.concatenate([wohi, wolo], axis=0).copy()               # (8,128,5)

    h0v = np.asarray(inputs["h0"], np.float32)[d].reshape(KC, 128).T.copy()
    c0v = np.asarray(inputs["c0"], np.float32)[d].reshape(KC, 128).T.copy()
    emb = np.ascontiguousarray(np.asarray(inputs["embed"], np.float32))
    return dict(emb=emb, idx=np.ascontiguousarray(idx), wihT=np.ascontiguousarray(wihT),
                bias=np.ascontiguousarray(bias), whh=whh, woT=woTb,
                h0v=h0v, c0v=c0v)


def _viterbi_host(feats, transitions):
    A = np.asarray(transitions, np.float32)
    C, L = 128, T // 128
    fm = feats.reshape(C, L, NT)
    Q = (np.where(np.eye(NT, dtype=bool)[None], 0.0, -3.0e4)
         * np.ones((C, 1, 1))).astype(np.float32)
    for s in range(L):
        M = A[None] + fm[:, s, :, None]
        Q = (M[:, :, :, None] + Q[:, None, :, :]).max(axis=2).astype(np.float32)
    init = np.full(NT, NEG, np.float32); init[START] = 0.0
    fv_starts = np.zeros((C, NT), np.float32)
    fv = init
    for c in range(C):
        fv_starts[c] = fv
        fv = (Q[c] + fv[None, :]).max(axis=1).astype(np.float32)
    bptrs = np.zeros((C, L, NT), np.int64)
    fvc = fv_starts.copy()
    for s in range(L):
        sc = fvc[:, None, :] + A[None]
        mx = sc.max(axis=2)
        bptrs[:, s] = np.argmax(sc, axis=2)
        fvc = (mx + fm[:, s]).astype(np.float32)
    fv_T = fvc[-1]
    bp = bptrs.reshape(T, NT)
    term = fv_T + A[STOP]
    best = int(np.argmax(term))
    score = np.float32(term[best])
    path = np.zeros(T, np.int32)
    cur = best
    for t in range(T - 1, -1, -1):
        path[t] = cur
        cur = bp[t][cur]
    return score, path


def kernel(**inputs):
    from concourse.bass_utils import run_bass_kernel_spmd

    if "k1" not in _cache:
        _cache["k1"] = _build_k1()
    nc = _cache["k1"]

    in_maps = [_prep_core(inputs, 0), _prep_core(inputs, 1)]
    trace = bool(int(os.environ.get("BASS_KERNEL_TRACE", "0")))
    res = run_bass_kernel_spmd(nc, in_maps, core_ids=[0, 1], trace=trace)
    kernel.last = res

    pf_f = res.results[0]["pf"]          # (5, T)
    pf_b = res.results[1]["pf"][:, ::-1]  # reverse time
    b_out = np.asarray(inputs["b_out"], np.float32)
    feats = (pf_f + pf_b + b_out[:, None]).T.astype(np.float32).copy()  # (T, 5)

    score, path = _viterbi_host(feats, np.asarray(inputs["transitions"], np.float32))
    sent_dtype = np.asarray(inputs["sentence"]).dtype
    return np.float32(score), path.astype(np.int32)


kernel.last = None


# revision 3
# speedup vs baseline: 1.0005x; 1.0005x over previous
"""BiLSTM-CRF Trainium2 kernel.

Strategy (2 of 8 cores do the heavy sequential work; the recurrence is
inherently serial per direction, so one core per direction):

  K1 (SPMD, cores 0..1; core0=forward, core1=backward via reversed inputs):
    - embedding gather (indirect DMA), PE transposes -> x^T
    - input projection xp^T = W_ihT^T... (fp32 matmuls), +bias, -> DRAM
    - 16384-step LSTM scan: W_hh in bf16 hi+lo split (bf16x2), h split
      hi+lo each step, fp32 PSUM accumulate -> near-fp32 accuracy at
      ~7.5us/step; output projection (w_out slice, bf16x2) fused into the
      scan -> partial feats pf (5, T)
  host: feats = pf_f + reverse(pf_b) + b_out
  Viterbi (host, fp32, same associativity as reference) + backtrace.

Self-contained: hardcodes all shapes from the problem spec.
"""
import os
import numpy as np
import ml_dtypes

V, E, H = 50000, 512, 1024
HH = H // 2           # 512 per direction
G = 4 * HH            # 2048 gates
KC = HH // 128        # 4 h-chunks
MC = G // 128         # 16 gate m-tiles
NT = 5
START, STOP = 3, 4
T = 16384
NEG = -10000.0
U = 16                # scan steps per For_i body
TB = 512              # phase-A T-block size

_cache = {}

# gate permutation: reference gate order is [i, f, g, o] in rows of w_ih/w_hh.
# device layout wants m-tiles ordered [i(0:4), f(4:8), o(8:12), g(12:16)].
def _perm():
    p = np.arange(G).reshape(4, HH)   # rows: i, f, g, o
    return np.concatenate([p[0], p[1], p[3], p[2]])  # i, f, o, g


def _build_k1():
    import concourse.bass as bass
    import concourse.mybir as mybir
    from concourse import bacc
    from concourse.bass import ds, ts
    from concourse.tile import TileContext
    from concourse.masks import make_identity

    f32 = mybir.dt.float32
    bf16 = mybir.dt.bfloat16
    u32 = mybir.dt.uint32

    nc = bacc.Bacc("TRN2", target_bir_lowering=False, debug=False, num_devices=2)

    emb_d = nc.dram_tensor("emb", [V, E], f32, kind="ExternalInput").ap()
    idx_d = nc.dram_tensor("idx", [128, T // 128], u32, kind="ExternalInput").ap()
    wihT_d = nc.dram_tensor("wihT", [KC, 128, G], f32, kind="ExternalInput").ap()
    bias_d = nc.dram_tensor("bias", [128, MC], f32, kind="ExternalInput").ap()
    whh_d = nc.dram_tensor("whh", [2 * KC, 128, G], bf16, kind="ExternalInput").ap()
    woT_d = nc.dram_tensor("woT", [2 * KC, 128, NT], bf16, kind="ExternalInput").ap()
    h0_d = nc.dram_tensor("h0v", [128, KC], f32, kind="ExternalInput").ap()
    c0_d = nc.dram_tensor("c0v", [128, KC], f32, kind="ExternalInput").ap()

    xpT_d = nc.dram_tensor("xpT", [MC, 128, T], f32, kind="Internal").ap()
    pf_d = nc.dram_tensor("pf", [NT, T], f32, kind="ExternalOutput").ap()

    with TileContext(nc) as tc:
        with (
            tc.tile_pool(name="const", bufs=1) as cpool,
            tc.tile_pool(name="wih", bufs=1) as wihpool,
            tc.tile_pool(name="whh", bufs=1) as whhpool,
            tc.tile_pool(name="state", bufs=1) as spool,
            tc.tile_pool(name="x", bufs=2) as xpool,
            tc.tile_pool(name="xt", bufs=2) as xtpool,
            tc.tile_pool(name="xpo", bufs=3) as xpopool,
            tc.tile_pool(name="xps", bufs=3) as xpspool,
            tc.tile_pool(name="pfs", bufs=3) as pfspool,
            tc.tile_pool(name="tmp", bufs=2) as tpool,
            tc.tile_pool(name="pst", bufs=2, space="PSUM") as pst,   # transposes
            tc.tile_pool(name="psx", bufs=2, space="PSUM") as psx,   # xp matmuls
            tc.tile_pool(name="psg", bufs=2, space="PSUM") as psg,   # scan gates
            tc.tile_pool(name="psf", bufs=2, space="PSUM") as psf,   # pf matmuls
        ):
            ident = cpool.tile([128, 128], f32, tag="ident")
            make_identity(nc, ident[:, :])
            idxt = cpool.tile([128, T // 128], u32, tag="idx")
            nc.sync.dma_start(out=idxt[:, :], in_=idx_d[:, :])
            biast = cpool.tile([128, MC], f32, tag="bias")
            nc.sync.dma_start(out=biast[:, :], in_=bias_d[:, :])

            wih = []
            for k in range(KC):
                w = wihpool.tile([128, G], f32, tag=f"wih{k}")
                nc.sync.dma_start(out=w[:, :], in_=wihT_d[k, :, :])
                wih.append(w)
            whh = []
            for k in range(2 * KC):
                w = whhpool.tile([128, G], bf16, tag=f"whh{k}")
                nc.sync.dma_start(out=w[:, :], in_=whh_d[k, :, :])
                whh.append(w)
            woT = cpool.tile([128, 2 * KC * NT], bf16, tag="woT")
            nc.sync.dma_start(
                out=woT[:, :].rearrange("p (w n) -> p w n", w=2 * KC),
                in_=woT_d[:, :, :].rearrange("w p n -> p w n"),
            )

            # ---------------- phase A: gather + transpose + xp ----------------
            for tb in range(T // TB):
                xts = []
                for e in range(KC):
                    xt = xtpool.tile([128, TB], f32, tag=f"xt{e}")
                    xts.append(xt)
                for sub in range(TB // 128):
                    b = tb * (TB // 128) + sub
                    xg = xpool.tile([128, E], f32, tag="xg")
                    nc.gpsimd.indirect_dma_start(
                        out=xg[:, :],
                        out_offset=None,
                        in_=emb_d[:, :],
                        in_offset=bass.IndirectOffsetOnAxis(ap=idxt[:, b:b + 1], axis=0),
                    )
                    for e in range(KC):
                        pt = pst.tile([128, 128], f32, tag="pt")
                        nc.tensor.transpose(pt[:, :], xg[:, ts(e, 128)], ident[:, :])
                        nc.vector.tensor_copy(xts[e][:, ts(sub, 128)], pt[:, :])
                for m in range(MC):
                    px = psx.tile([128, TB], f32, tag="px")
                    for e in range(KC):
                        nc.tensor.matmul(
                            px[:, :], wih[e][:, ts(m, 128)], xts[e][:, :],
                            start=(e == 0), stop=(e == KC - 1),
                        )
                    xo = xpopool.tile([128, TB], f32, tag="xo")
                    nc.vector.tensor_scalar(
                        out=xo[:, :], in0=px[:, :], scalar1=biast[:, m:m + 1],
                        scalar2=None, op0=mybir.AluOpType.add,
                    )
                    nc.sync.dma_start(out=xpT_d[m, :, ts(tb, TB)], in_=xo[:, :])

            # ---------------- scan ----------------
            c_t = spool.tile([128, KC], f32, tag="c")
            nc.sync.dma_start(out=c_t[:, :], in_=c0_d[:, :])
            hbuf = spool.tile([128, KC * (U + 1)], f32, tag="hbuf")
            nc.sync.dma_start(out=hbuf[:, 0:KC], in_=h0_d[:, :])
            hbf = spool.tile([128, 2 * KC * (U + 1)], bf16, tag="hbf")
            s0 = hbf[:, 0:2 * KC].rearrange("p (k two) -> p k two", two=2)
            h0r = hbuf[:, 0:KC].rearrange("p (k one) -> p k one", one=1)
            nc.vector.tensor_copy(s0[:, :, 0:1], h0r)
            tmp0 = tpool.tile([128, KC], f32, tag="hlo")
            tmp0r = tmp0[:, :].rearrange("p (k one) -> p k one", one=1)
            nc.vector.tensor_tensor(tmp0r, h0r, s0[:, :, 0:1], mybir.AluOpType.subtract)
            nc.vector.tensor_copy(s0[:, :, 1:2], tmp0r)

            with tc.For_i(0, T, U) as it:
                xpt = xpspool.tile([128, MC * U], f32, tag="xpt")
                nc.sync.dma_start(
                    out=xpt[:, :].rearrange("q (j t) -> q j t", j=MC),
                    in_=xpT_d[:, :, ds(it, U)].rearrange("j q t -> q j t"),
                )
                for s in range(U):
                    hin = hbf[:, 2 * KC * s:2 * KC * (s + 1)]
                    g = psg.tile([128, 2 * MC], f32, tag="g")
                    for m in range(MC):
                        for wv in range(2):
                            for k in range(KC):
                                nc.tensor.matmul(
                                    g[:, 2 * m:2 * m + 2],
                                    whh[wv * KC + k][:, ts(m, 128)],
                                    hin[:, 2 * k:2 * k + 2],
                                    start=(wv == 0 and k == 0),
                                    stop=(wv == 1 and k == KC - 1),
                                )
                    a = tpool.tile([128, MC], f32, tag="a")
                    gr = g[:, :].rearrange("p (m two) -> p m two", two=2)
                    ar = a[:, :].rearrange("p (m one) -> p m one", one=1)
                    xr = xpt[:, :].rearrange("q (j t) -> q j t", j=MC)[:, :, s:s + 1]
                    nc.vector.tensor_tensor(ar, gr[:, :, 0:1], xr, mybir.AluOpType.add)
                    nc.vector.tensor_tensor(ar, ar, gr[:, :, 1:2], mybir.AluOpType.add)
                    act = tpool.tile([128, MC], f32, tag="act")
                    nc.scalar.activation(act[:, 0:12], a[:, 0:12],
                                         mybir.ActivationFunctionType.Sigmoid)
                    nc.scalar.activation(act[:, 12:16], a[:, 12:16],
                                         mybir.ActivationFunctionType.Tanh)
                    ig = tpool.tile([128, KC], f32, tag="ig")
                    nc.vector.tensor_tensor(ig[:, :], act[:, 0:4], act[:, 12:16],
                                            mybir.AluOpType.mult)
                    nc.vector.tensor_tensor(c_t[:, :], act[:, 4:8], c_t[:, :],
                                            mybir.AluOpType.mult)
                    nc.vector.tensor_tensor(c_t[:, :], c_t[:, :], ig[:, :],
                                            mybir.AluOpType.add)
                    tct = tpool.tile([128, KC], f32, tag="tc")
                    nc.scalar.activation(tct[:, :], c_t[:, :],
                                         mybir.ActivationFunctionType.Tanh)
                    hout = hbuf[:, KC * (s + 1):KC * (s + 2)]
                    nc.vector.tensor_tensor(hout, act[:, 8:12], tct[:, :],
                                            mybir.AluOpType.mult)
                    sl = hbf[:, 2 * KC * (s + 1):2 * KC * (s + 2)].rearrange(
                        "p (k two) -> p k two", two=2)
                    hor = hout.rearrange("p (k one) -> p k one", one=1)
                    nc.vector.tensor_copy(sl[:, :, 0:1], hor)
                    hlo = tpool.tile([128, KC], f32, tag="hlo")
                    hlor = hlo[:, :].rearrange("p (k one) -> p k one", one=1)
                    nc.vector.tensor_tensor(hlor, hor, sl[:, :, 0:1],
                                            mybir.AluOpType.subtract)
                    nc.vector.tensor_copy(sl[:, :, 1:2], hlor)
                # fused output projection for these U steps
                pp = psf.tile([NT, 2 * U], f32, tag="pp")
                rhs = hbf[:, 2 * KC:].rearrange("p (s kk) -> p s kk", kk=2 * KC)
                for wv in range(2):
                    for k in range(KC):
                        nc.tensor.matmul(
                            pp[:, :],
                            woT[:, :].rearrange("p (w n) -> p w n", w=2 * KC)[:, wv * KC + k, :],
                            rhs[:, 0:U, 2 * k:2 * k + 2],
                            start=(wv == 0 and k == 0),
                            stop=(wv == 1 and k == KC - 1),
                        )
                pft = pfspool.tile([NT, 2 * U], f32, tag="pft")
                nc.vector.tensor_copy(pft[:, :], pp[:, :])
                pfo = pfspool.tile([NT, U], f32, tag="pfo")
                pfr = pft[:, :].rearrange("p (s two) -> p s two", two=2)
                nc.vector.tensor_tensor(
                    pfo[:, :].rearrange("p (s one) -> p s one", one=1),
                    pfr[:, :, 0:1], pfr[:, :, 1:2], mybir.AluOpType.add)
                nc.sync.dma_start(out=pf_d[:, ds(it, U)], in_=pfo[:, :])
                # carry h to slot 0
                nc.vector.tensor_copy(hbuf[:, 0:KC], hbuf[:, KC * U:KC * (U + 1)])
                nc.vector.tensor_copy(hbf[:, 0:2 * KC], hbf[:, 2 * KC * U:2 * KC * (U + 1)])
    nc.compile()
    return nc


def _prep_core(inputs, d):
    """Per-core (direction d: 0=fwd, 1=bwd) input dict for K1."""
    perm = _perm()
    sfx = "f" if d == 0 else "b"
    sent = np.asarray(inputs["sentence"]).astype(np.uint32)
    if d == 1:
        sent = sent[::-1].copy()
    idx = sent.reshape(T // 128, 128).T.copy()  # [p, b] = sent[b*128+p]

    w_ih = np.asarray(inputs[f"w_ih_{sfx}"], np.float32)[perm]     # (G, E)
    wihT = w_ih.T.reshape(KC, 128, G).copy()                        # [e,128,G]
    bias = (np.asarray(inputs[f"b_ih_{sfx}"], np.float32) +
            np.asarray(inputs[f"b_hh_{sfx}"], np.float32))[perm]
    bias = bias.reshape(MC, 128).T.copy()                           # (128, MC)

    w_hh = np.asarray(inputs[f"w_hh_{sfx}"], np.float32)[perm]      # (G, HH)
    whhT = w_hh.T.reshape(KC, 128, G)                               # [k,128,G]
    hi = whhT.astype(ml_dtypes.bfloat16)
    lo = (whhT - hi.astype(np.float32)).astype(ml_dtypes.bfloat16)
    whh = np.concatenate([hi, lo], axis=0).copy()                   # (8,128,G)

    wo = np.asarray(inputs["w_out"], np.float32)[:, d * HH:(d + 1) * HH]  # (5, HH)
    woT = wo.T.reshape(KC, 128, NT)                                  # [k,128,5]
    wohi = woT.astype(ml_dtypes.bfloat16)
    wolo = (woT - wohi.astype(np.float32)).astype(ml_dtypes.bfloat16)
    woTb = np.concatenate([wohi, wolo], axis=0).copy()               # (8,128,5)

    h0v = np.asarray(inputs["h0"], np.float32)[d].reshape(KC, 128).T.copy()
    c0v = np.asarray(inputs["c0"], np.float32)[d].reshape(KC, 128).T.copy()
    emb = np.ascontiguousarray(np.asarray(inputs["embed"], np.float32))
    return dict(emb=emb, idx=np.ascontiguousarray(idx), wihT=np.ascontiguousarray(wihT),
                bias=np.ascontiguousarray(bias), whh=whh, woT=woTb,
                h0v=h0v, c0v=c0v)


def _viterbi_host(feats, transitions):
    A = np.asarray(transitions, np.float32)
    C, L = 128, T // 128
    fm = feats.reshape(C, L, NT)
    Q = (np.where(np.eye(NT, dtype=bool)[None], 0.0, -3.0e4)
         * np.ones((C, 1, 1))).astype(np.float32)
    for s in range(L):
        M = A[None] + fm[:, s, :, None]
        Q = (M[:, :, :, None] + Q[:, None, :, :]).max(axis=2).astype(np.float32)
    init = np.full(NT, NEG, np.float32); init[START] = 0.0
    fv_starts = np.zeros((C, NT), np.float32)
    fv = init
    ksum = 0.0  # float64 accumulator of removed shifts
    kof = np.zeros(C, np.float64)
    for c in range(C):
        fv_starts[c] = fv
        kof[c] = ksum
        fv = (Q[c] + fv[None, :]).max(axis=1).astype(np.float32)
        k = np.float32(fv.max())
        fv = (fv - k).astype(np.float32)
        ksum += float(k)
    bptrs = np.zeros((C, L, NT), np.int64)
    fvc = fv_starts.copy()
    for s in range(L):
        sc = fvc[:, None, :] + A[None]
        mx = sc.max(axis=2)
        bptrs[:, s] = np.argmax(sc, axis=2)
        fvc = (mx + fm[:, s]).astype(np.float32)
    fv_T = fvc[-1]
    bp = bptrs.reshape(T, NT)
    term = fv_T + A[STOP]
    best = int(np.argmax(term))
    score = np.float32(float(term[best]) + float(kof[-1]))
    path = np.zeros(T, np.int32)
    cur = best
    for t in range(T - 1, -1, -1):
        path[t] = cur
        cur = bp[t][cur]
    return score, path


def kernel(**inputs):
    from concourse.bass_utils import run_bass_kernel_spmd

    if "k1" not in _cache:
        _cache["k1"] = _build_k1()
    nc = _cache["k1"]

    in_maps = [_prep_core(inputs, 0), _prep_core(inputs, 1)]
    trace = bool(int(os.environ.get("BASS_KERNEL_TRACE", "0")))
    res = run_bass_kernel_spmd(nc, in_maps, core_ids=[0, 1], trace=trace)
    kernel.last = res

    pf_f = res.results[0]["pf"]          # (5, T)
    pf_b = res.results[1]["pf"][:, ::-1]  # reverse time
    b_out = np.asarray(inputs["b_out"], np.float32)
    feats = (pf_f + pf_b + b_out[:, None]).T.astype(np.float32).copy()  # (T, 5)

    score, path = _viterbi_host(feats, np.asarray(inputs["transitions"], np.float32))
    sent_dtype = np.asarray(inputs["sentence"]).dtype
    return np.float32(score), path.astype(np.int32)


kernel.last = None
